# revision 5
# baseline (speedup 1.0000x reference)
"""Causal multi-head attention on 8 Trainium2 NeuronCores.

Problem: residual_stream [4, 2048, 1024] fp32, per-head QKV weights
[16, 1024, 64], output projection [1024, 1024]; causal softmax attention.

Sharding: tensor-parallel over heads — core c computes heads (2c, 2c+1)
for all 4 batches, applies the matching 128-row slice of weight_out, and
returns a full-shape partial output; the host sums the 8 partials
(all-reduce equivalent).

Per-core pipeline (all matmuls in f32r — full-rate fp32 mode):
  1. Q^T/K^T/V^T projections: stationary = weight m-tile, moving = X^T.
  2. V^T -> V via PE transpose; V augmented with a ones column so the
     P@V matmul also emits softmax row-sums for free.
  3. S^T = (Q K^T)^T blockwise, lower-triangle blocks only; the two
     heads are packed as concurrent K=64 row-group matmuls.
  4. P^T = exp(S^T/8) on ACT straight out of PSUM (scores are O(10), so
     no max-subtraction is needed); diagonal blocks get a 0/1 mask.
  5. O_aug^T = V_aug^T P^T accumulated over k-tiles: rows 0:64 = O^T,
     row 64 = row-sums. Normalization: sums row -> partition 0 (DVE
     cross-quadrant copy), K=1 outer-product broadcast on PE,
     reciprocal + multiply on DVE.
  6. Y_partial = O_norm^T.T @ W_out[128c:128c+128] -> DRAM.
"""
import sys
import types

sys.path.insert(0, "/opt/trn_rl_repo")

import numpy as np

import concourse.bass as bass
import concourse.tile as tile
from concourse import mybir

F32 = mybir.dt.float32
F32R = mybir.dt.float32r

B = 4
SEQ = 2048
DM = 1024
DH = 64
NH = 16
NCORES = 8
HPC = NH // NCORES          # heads per core = 2
MT = DM // 128              # m-tiles = 8
KT = SEQ // 128             # k-tiles = 16
QC = SEQ // 512             # q-chunks of 512 = 4

_CACHE = {}


def _split_waits(d, max_waits=1):
    # This walrus build allows a single sync-wait on several instruction
    # encodings (CTRL Drain, fused-LDW f32 Matmult). Hoist excess waits
    # onto same-engine NoOp carriers directly in the BIR JSON.
    for fn in d.get("functions", []):
        for blk in fn.get("blocks", []):
            out = []
            for inst in blk.get("instructions", []):
                si = inst.get("sync_info") or {}
                waits = si.get("on_wait") or []
                if len(waits) > max_waits:
                    extra = waits[: len(waits) - max_waits]
                    rest = waits[len(waits) - max_waits:]
                    for i, w in enumerate(extra):
                        out.append({
                            "name": f"{inst['name']}_sw{i}",
                            "opcode": "NoOp",
                            "engine": inst["engine"],
                            "ins": [],
                            "outs": [],
                            "sync_info": {"on_update": [], "on_wait": [w]},
                        })
                    inst = dict(inst)
                    inst["sync_info"] = {
                        "on_update": list(si.get("on_update") or []),
                        "on_wait": rest,
                    }
                out.append(inst)
            blk["instructions"] = out
    return d


def _patch_nc(nc):
    import orjson

    def to_json_bytes(self):
        return orjson.dumps(
            _split_waits(orjson.loads(mybir.module_to_json_bytes(self.m)))
        )

    nc.to_json_bytes = types.MethodType(to_json_bytes, nc)
    return nc


def _build_nc():
    nc = bass.Bass()

    xt = nc.dram_tensor("xt", [B, DM, SEQ], F32R, kind="ExternalInput")
    w_all = nc.dram_tensor("w_all", [128, MT, 3 * 128], F32R, kind="ExternalInput")
    wout = nc.dram_tensor("wout", [128, DM], F32R, kind="ExternalInput")
    masks = nc.dram_tensor("masks", [4, 128, 512], F32, kind="ExternalInput")
    ident = nc.dram_tensor("ident", [128, 128], F32, kind="ExternalInput")
    ones = nc.dram_tensor("ones", [1, 64], F32R, kind="ExternalInput")
    y = nc.dram_tensor("y", [B, SEQ, DM], F32, kind="ExternalOutput")

    with tile.TileContext(nc) as tc:
        with (
            tc.tile_pool(name="const", bufs=1) as const,
            tc.tile_pool(name="xtp", bufs=1) as xtp,
            tc.tile_pool(name="qkv", bufs=1) as qkv,
            tc.tile_pool(name="ptp", bufs=16) as ptp,
            tc.tile_pool(name="onp", bufs=1) as onp,
            tc.tile_pool(name="small", bufs=4) as small,
            tc.tile_pool(name="yp", bufs=2) as yp,
            tc.tile_pool(name="ps", bufs=8, space="PSUM") as ps,
        ):
            w_t = const.tile([128, MT, 3 * 128], F32R, tag="w")
            nc.sync.dma_start(out=w_t, in_=w_all[:])
            wout_t = const.tile([128, DM], F32R, tag="wout")
            nc.sync.dma_start(out=wout_t, in_=wout[:])
            mask_t = const.tile([128, 4, 512], F32, tag="mask")
            nc.sync.dma_start(
                out=mask_t, in_=masks[:].rearrange("j p f -> p j f")
            )
            ident_t = const.tile([128, 128], F32, tag="ident")
            nc.sync.dma_start(out=ident_t, in_=ident[:])
            ones_t = const.tile([1, 64], F32R, tag="ones")
            nc.sync.dma_start(out=ones_t, in_=ones[:])

            for b in range(B):
                # ---- QKV projections -> Q^T/K^T [d-stack, seq] (f32r), V^T (f32)
                xt_t = xtp.tile([128, MT, SEQ], F32R, tag="xt")
                nc.sync.dma_start(
                    out=xt_t, in_=xt[b].rearrange("(m p) q -> p m q", p=128)
                )
                qt = qkv.tile([128, SEQ], F32R, tag="qt")
                kt = qkv.tile([128, SEQ], F32R, tag="kt")
                vt = qkv.tile([128, SEQ], F32, tag="vt")
                for cq in range(QC):
                    cqs = slice(cq * 512, cq * 512 + 512)
                    pq = ps.tile([128, 512], F32, tag="bank")
                    pk = ps.tile([128, 512], F32, tag="bank")
                    pv = ps.tile([128, 512], F32, tag="bank")
                    for m in range(MT):
                        for proj, dst in ((0, pq), (1, pk), (2, pv)):
                            nc.tensor.matmul(
                                dst[:],
                                w_t[:, m, proj * 128:(proj + 1) * 128],
                                xt_t[:, m, cqs],
                                start=(m == 0),
                                stop=(m == MT - 1),
                            )
                    nc.scalar.copy(qt[:, cqs], pq[:])
                    nc.scalar.copy(kt[:, cqs], pk[:])
                    nc.vector.tensor_copy(vt[:, cqs], pv[:])

                # ---- V^T -> V_aug [k, head, d+1] via PE transpose
                vaug = qkv.tile([128, KT, HPC, 65], F32R, tag="vaug")
                ones_src = bass.AP(
                    tensor=ones[:].tensor,
                    offset=ones[:].offset,
                    ap=[[0, 128], [2, KT], [1, HPC], [1, 1]],
                )
                nc.gpsimd.dma_start(out=vaug[:, :, :, 64:65], in_=ones_src)
                for tk in range(KT):
                    pt_ps = ps.tile([128, 128], F32, tag="bank")
                    nc.tensor.transpose(
                        pt_ps[:], vt[:, tk * 128:(tk + 1) * 128], ident_t[:]
                    )
                    nc.scalar.copy(vaug[:, tk, 0, 0:64], pt_ps[:, 0:64])
                    nc.scalar.copy(vaug[:, tk, 1, 0:64], pt_ps[:, 64:128])

                # ---- attention per q-chunk; heads packed as K=64 row groups.
                # PV is software-pipelined one k-tile behind S^T/exp so only
                # ~2 P^T tiles per head are ever live.
                onorm = onp.tile([128, SEQ], F32R, tag="onorm")
                for cq in range(QC):
                    cqs = slice(cq * 512, cq * 512 + 512)
                    ntk = 4 * cq + 4
                    ops = {h: ps.tile([65, 512], F32, tag="bank",
                                      name=f"ops_b{b}c{cq}h{h}")
                           for h in (0, 1)}

                    def make_pt(tk):
                        tks = slice(tk * 128, tk * 128 + 128)
                        st = {}
                        for h in (0, 1):
                            hs = slice(h * 64, h * 64 + 64)
                            spsum = ps.tile([128, 512], F32, tag="bank")
                            nc.tensor.matmul(
                                spsum[:], kt[hs, tks], qt[hs, cqs],
                                start=True, stop=True,
                            )
                            st[h] = spsum
                        pts = {}
                        for h in (0, 1):
                            pt = ptp.tile([128, 512], F32R, tag="pt")
                            nc.scalar.activation(
                                pt[:], st[h][:],
                                mybir.ActivationFunctionType.Exp,
                                bias=0.0, scale=0.125,
                            )
                            if tk >= 4 * cq:
                                nc.vector.tensor_mul(
                                    pt[:], pt.bitcast(F32)[:],
                                    mask_t[:, tk - 4 * cq, :],
                                )
                            pts[h] = pt
                        return pts

                    def pv_step(tk, pts):
                        for h in (0, 1):
                            nc.tensor.matmul(
                                ops[h][:], vaug[:, tk, h, :], pts[h][:],
                                start=(tk == 0), stop=(tk == ntk - 1),
                            )

                    prev = make_pt(0)
                    for tk in range(1, ntk):
                        cur = make_pt(tk)
                        pv_step(tk - 1, prev)
                        prev = cur
                    pv_step(ntk - 1, prev)

                    for h in (0, 1):
                        stage = small.tile([1, 512], F32R, tag="sums")
                        nc.vector.tensor_copy(stage[0:1, :], ops[h][64:65, :])
                        bc = ps.tile([64, 512], F32, tag="bank")
                        nc.tensor.matmul(
                            bc[:], ones_t[:], stage[0:1, :],
                            start=True, stop=True,
                        )
                        rec = small.tile([64, 512], F32, tag="rec")
                        nc.vector.reciprocal(rec[:], bc[:])
                        nc.vector.tensor_mul(
                            onorm[h * 64:(h + 1) * 64, cqs],
                            ops[h][0:64, :], rec[:],
                        )

                # ---- output projection: Y[b] = O_norm^T.T @ W_out slice
                for qi in range(KT):
                    ysb = yp.tile([128, DM], F32, tag="y")
                    for nh in range(2):
                        yps = ps.tile([128, 512], F32, tag="bank")
                        nc.tensor.matmul(
                            yps[:],
                            onorm[:, qi * 128:(qi + 1) * 128],
                            wout_t[:, nh * 512:(nh + 1) * 512],
                            start=True, stop=True,
                        )
                        if nh == 0:
                            nc.scalar.copy(ysb[:, 0:512], yps[:])
                        else:
                            nc.vector.tensor_copy(ysb[:, 512:1024], yps[:])
                    nc.sync.dma_start(
                        out=y[b, qi * 128:(qi + 1) * 128, :], in_=ysb
                    )

    return _patch_nc(nc)


def _causal_masks():
    m = np.zeros((4, 128, 512), np.float32)
    i = np.arange(128)[:, None]
    f = np.arange(512)[None, :]
    for j in range(4):
        m[j] = (f >= i + 128 * j).astype(np.float32)
    return m


def _prepare_in_maps(residual_stream, weight_query, weight_key, weight_value,
                     weight_out):
    xt = np.ascontiguousarray(
        np.asarray(residual_stream, np.float32).transpose(0, 2, 1)
    )
    masks = _causal_masks()
    ident = np.eye(128, dtype=np.float32)
    ones = np.ones((1, 64), np.float32)
    in_maps = []
    for c in range(NCORES):
        w = np.empty((128, MT, 3 * 128), np.float32)
        for proj, wt in ((0, weight_query), (1, weight_key), (2, weight_value)):
            # [1024, 128]: column h*64+d for core-local head h
            wc = np.asarray(wt[HPC * c:HPC * (c + 1)], np.float32)
            wc = wc.transpose(1, 0, 2).reshape(DM, HPC * DH)
            w[:, :, proj * 128:(proj + 1) * 128] = (
                wc.reshape(MT, 128, HPC * DH).transpose(1, 0, 2)
            )
        wo = np.ascontiguousarray(
            np.asarray(weight_out, np.float32)[128 * c:128 * (c + 1), :]
        )
        in_maps.append({
            "xt": xt,
            "w_all": np.ascontiguousarray(w),
            "wout": wo,
            "masks": masks,
            "ident": ident,
            "ones": ones,
        })
    return in_maps


def kernel(residual_stream, weight_query, weight_key, weight_value,
           weight_out, trace=False):
    from concourse.bass_utils import run_bass_kernel_spmd

    if "nc" not in _CACHE:
        _CACHE["nc"] = _build_nc()
    nc = _CACHE["nc"]

    in_maps = _prepare_in_maps(
        residual_stream, weight_query, weight_key, weight_value, weight_out
    )
    res = run_bass_kernel_spmd(
        nc, in_maps, list(range(NCORES)), trace=trace
    )
    _CACHE["last_result"] = res
    out = np.zeros((B, SEQ, DM), np.float32)
    for c in range(NCORES):
        out += res.results[c]["y"]
    return out


# revision 7
# speedup vs baseline: 1.2032x; 1.2032x over previous
"""Causal multi-head attention on 8 Trainium2 NeuronCores.

Problem: residual_stream [4, 2048, 1024] fp32, per-head QKV weights
[16, 1024, 64], output projection [1024, 1024]; causal softmax attention.

Sharding: tensor-parallel over heads — core c computes heads (2c, 2c+1)
for all 4 batches, applies the matching 128-row slice of weight_out, and
returns a full-shape partial output; the host sums the 8 partials
(all-reduce equivalent).

Per-core pipeline (all matmuls in f32r — full-rate fp32 mode):
  1. Q^T/K^T/V^T projections: stationary = weight m-tile, moving = X^T.
  2. V^T -> V via PE transpose; V augmented with a ones column so the
     P@V matmul also emits softmax row-sums for free.
  3. S^T = (Q K^T)^T blockwise, lower-triangle blocks only; the two
     heads are packed as concurrent K=64 row-group matmuls.
  4. P^T = exp(S^T/8) on ACT straight out of PSUM (scores are O(10), so
     no max-subtraction is needed); diagonal blocks get a 0/1 mask.
     PV runs two k-tiles behind S^T/exp (software pipeline).
  5. O_aug^T = V_aug^T P^T accumulated over k-tiles: rows 0:64 = O^T,
     row 64 = row-sums. Sums rows are staged to partitions 0..7, one
     batched reciprocal per batch, broadcast back via a DRAM bounce,
     then a single in-place multiply per (chunk, head).
  6. Y_partial = O_norm^T.T @ W_out[128c:128c+128] -> DRAM.

Phase order interleaves batch b's normalize/Wout tail behind batch
b+1's projections to keep the PE dense (HAM stays at full clock).
"""
import sys
import types

sys.path.insert(0, "/opt/trn_rl_repo")

import numpy as np

import concourse.bass as bass
import concourse.tile as tile
from concourse import mybir

F32 = mybir.dt.float32
F32R = mybir.dt.float32r

B = 4
SEQ = 2048
DM = 1024
DH = 64
NH = 16
NCORES = 8
HPC = NH // NCORES          # heads per core = 2
MT = DM // 128              # m-tiles = 8
KT = SEQ // 128             # k-tiles = 16
QC = SEQ // 512             # q-chunks of 512 = 4

_CACHE = {}


def _split_waits(d, max_waits=1):
    # This walrus build allows a single sync-wait on several instruction
    # encodings (CTRL Drain, fused-LDW f32 Matmult). Hoist excess waits
    # onto same-engine NoOp carriers directly in the BIR JSON.
    for fn in d.get("functions", []):
        for blk in fn.get("blocks", []):
            out = []
            for inst in blk.get("instructions", []):
                si = inst.get("sync_info") or {}
                waits = si.get("on_wait") or []
                if len(waits) > max_waits:
                    extra = waits[: len(waits) - max_waits]
                    rest = waits[len(waits) - max_waits:]
                    for i, w in enumerate(extra):
                        out.append({
                            "name": f"{inst['name']}_sw{i}",
                            "opcode": "NoOp",
                            "engine": inst["engine"],
                            "ins": [],
                            "outs": [],
                            "sync_info": {"on_update": [], "on_wait": [w]},
                        })
                    inst = dict(inst)
                    inst["sync_info"] = {
                        "on_update": list(si.get("on_update") or []),
                        "on_wait": rest,
                    }
                out.append(inst)
            blk["instructions"] = out
    return d


def _patch_nc(nc):
    import orjson

    def to_json_bytes(self):
        return orjson.dumps(
            _split_waits(orjson.loads(mybir.module_to_json_bytes(self.m)))
        )

    nc.to_json_bytes = types.MethodType(to_json_bytes, nc)
    return nc


def _build_nc():
    nc = bass.Bass()

    xt = nc.dram_tensor("xt", [B, DM, SEQ], F32R, kind="ExternalInput")
    w_all = nc.dram_tensor("w_all", [128, MT, 3 * 128], F32R, kind="ExternalInput")
    wout = nc.dram_tensor("wout", [128, DM], F32R, kind="ExternalInput")
    masks = nc.dram_tensor("masks", [4, 128, 512], F32, kind="ExternalInput")
    ident = nc.dram_tensor("ident", [128, 128], F32, kind="ExternalInput")
    ones = nc.dram_tensor("ones", [1, 64], F32R, kind="ExternalInput")
    y = nc.dram_tensor("y", [B, SEQ, DM], F32, kind="ExternalOutput")
    rscr = nc.dram_tensor("rscr", [B, 2 * QC, 512], F32)  # recip bounce

    with tile.TileContext(nc) as tc:
        with (
            tc.tile_pool(name="const", bufs=1) as const,
            tc.tile_pool(name="xtp", bufs=1) as xtp,
            tc.tile_pool(name="qkp", bufs=2) as qkp,
            tc.tile_pool(name="vtp", bufs=1) as vtp,
            tc.tile_pool(name="ptp", bufs=8) as ptp,
            tc.tile_pool(name="onp", bufs=1) as onp,
            tc.tile_pool(name="small", bufs=2) as small,
            tc.tile_pool(name="bcp", bufs=2) as bcp,
            tc.tile_pool(name="yp", bufs=2) as yp,
            tc.tile_pool(name="ps", bufs=8, space="PSUM") as ps,
        ):
            w_t = const.tile([128, MT, 3 * 128], F32R, tag="w")
            nc.sync.dma_start(out=w_t, in_=w_all[:])
            wout_t = const.tile([128, DM], F32R, tag="wout")
            nc.sync.dma_start(out=wout_t, in_=wout[:])
            mask_t = const.tile([128, 4, 512], F32, tag="mask")
            nc.sync.dma_start(
                out=mask_t, in_=masks[:].rearrange("j p f -> p j f")
            )
            ident_t = const.tile([128, 128], F32, tag="ident")
            nc.sync.dma_start(out=ident_t, in_=ident[:])

            # V_aug allocated once; ones column filled once.
            vaug = const.tile([128, KT, HPC, 65], F32R, tag="vaug")
            ones_src = bass.AP(
                tensor=ones[:].tensor,
                offset=ones[:].offset,
                ap=[[0, 128], [2, KT], [1, HPC], [1, 1]],
            )
            nc.gpsimd.dma_start(out=vaug[:, :, :, 64:65], in_=ones_src)

            def proj_and_transpose(b, name):
                """QKV projections + V transpose for batch b."""
                xt_t = xtp.tile([128, MT, SEQ], F32R, tag="xt",
                                name=f"xt_{name}")
                xv = xt[b].rearrange("(m p) q -> p m q", p=128)
                for m in range(MT):
                    nc.sync.dma_start(out=xt_t[:, m, :], in_=xv[:, m, :])
                qt = qkp.tile([128, SEQ], F32R, tag="qt", name=f"qt_{name}")
                kt = qkp.tile([128, SEQ], F32R, tag="kt", name=f"kt_{name}")
                vt = vtp.tile([128, SEQ], F32, tag="vt", name=f"vt_{name}")
                for cq in range(QC):
                    cqs = slice(cq * 512, cq * 512 + 512)
                    pq = ps.tile([128, 512], F32, tag="bank",
                                 name=f"pq_{name}{cq}")
                    pk = ps.tile([128, 512], F32, tag="bank",
                                 name=f"pk_{name}{cq}")
                    pv = ps.tile([128, 512], F32, tag="bank",
                                 name=f"pv_{name}{cq}")
                    for m in range(MT):
                        for proj, dst in ((0, pq), (1, pk), (2, pv)):
                            nc.tensor.matmul(
                                dst[:],
                                w_t[:, m, proj * 128:(proj + 1) * 128],
                                xt_t[:, m, cqs],
                                start=(m == 0),
                                stop=(m == MT - 1),
                            )
                    nc.scalar.copy(qt[:, cqs], pq[:])
                    nc.scalar.copy(kt[:, cqs], pk[:])
                    nc.vector.tensor_copy(vt[:, cqs], pv[:])
                # V^T -> V_aug via PE transpose (both heads per shot)
                for tk in range(KT):
                    pt_ps = ps.tile([128, 128], F32, tag="bank",
                                    name=f"tp_{name}{tk}")
                    nc.tensor.transpose(
                        pt_ps[:], vt[:, tk * 128:(tk + 1) * 128], ident_t[:]
                    )
                    nc.vector.tensor_copy(vaug[:, tk, 0, 0:64], pt_ps[:, 0:64])
                    nc.vector.tensor_copy(vaug[:, tk, 1, 0:64], pt_ps[:, 64:128])
                return qt, kt

            def attention(b, qt, kt, onorm, stage_all, scr64):
                for cq in range(QC):
                    cqs = slice(cq * 512, cq * 512 + 512)
                    ntk = 4 * cq + 4
                    ops = {h: ps.tile([65, 512], F32, tag="bank",
                                      name=f"ops_b{b}c{cq}h{h}")
                           for h in (0, 1)}

                    def make_pt(tk):
                        tks = slice(tk * 128, tk * 128 + 128)
                        st = {}
                        for h in (0, 1):
                            hs = slice(h * 64, h * 64 + 64)
                            spsum = ps.tile([128, 512], F32, tag="bank",
                                            name=f"s_b{b}c{cq}t{tk}h{h}")
                            nc.tensor.matmul(
                                spsum[:], kt[hs, tks], qt[hs, cqs],
                                start=True, stop=True,
                            )
                            st[h] = spsum
                        pts = {}
                        for h in (0, 1):
                            pt = ptp.tile([128, 512], F32R, tag="pt",
                                          name=f"pt_b{b}c{cq}t{tk}h{h}")
                            nc.scalar.activation(
                                pt[:], st[h][:],
                                mybir.ActivationFunctionType.Exp,
                                bias=0.0, scale=0.125,
                            )
                            if tk >= 4 * cq:
                                nc.vector.tensor_mul(
                                    pt[:], pt.bitcast(F32)[:],
                                    mask_t[:, tk - 4 * cq, :],
                                )
                            pts[h] = pt
                        return pts

                    def pv_step(tk, pts):
                        for h in (0, 1):
                            nc.tensor.matmul(
                                ops[h][:], vaug[:, tk, h, :], pts[h][:],
                                start=(tk == 0), stop=(tk == ntk - 1),
                            )

                    pend = []
                    for tk in range(ntk):
                        pend.append((tk, make_pt(tk)))
                        if len(pend) > 2:
                            pv_step(*pend.pop(0))
                    while pend:
                        pv_step(*pend.pop(0))

                    for h in (0, 1):
                        p = 2 * cq + h
                        # sums row -> partition p of stage_all (2 hops)
                        nc.scalar.copy(scr64[64:65, :], ops[h][64:65, :])
                        nc.sync.dma_start(
                            out=stage_all[p:p + 1, :], in_=scr64[64:65, :]
                        )
                        # evacuate O^T rows (unnormalized)
                        if h == 0:
                            nc.scalar.copy(
                                onorm[0:64, cqs], ops[h][0:64, :]
                            )
                        else:
                            nc.vector.tensor_copy(
                                onorm[64:128, cqs], ops[h][0:64, :]
                            )

            def normalize(b, onorm, stage_all):
                recip = small.tile([2 * QC, 512], F32, tag="recip",
                                   name=f"recip_{b}")
                nc.vector.reciprocal(recip[:], stage_all[:])
                nc.sync.dma_start(out=rscr[b], in_=recip[:])
                for cq in range(QC):
                    cqs = slice(cq * 512, cq * 512 + 512)
                    for h in (0, 1):
                        p = 2 * cq + h
                        bc = bcp.tile([128, 512], F32, tag="bc",
                                      name=f"bc_b{b}c{cq}h{h}")
                        hs = slice(h * 64, h * 64 + 64)
                        bc_src = bass.AP(
                            tensor=rscr[:].tensor,
                            offset=(b * 2 * QC + p) * 512,
                            ap=[[0, 64], [1, 512]],
                        )
                        nc.gpsimd.dma_start(out=bc[hs, :], in_=bc_src)
                        nc.vector.tensor_mul(
                            onorm[hs, cqs], onorm.bitcast(F32)[hs, cqs],
                            bc[hs, :],
                        )

            def wout_phase(b, onorm):
                for qi in range(KT):
                    ysb = yp.tile([128, DM], F32, tag="y", name=f"y_{b}_{qi}")
                    for nh in range(2):
                        yps = ps.tile([128, 512], F32, tag="bank",
                                      name=f"yps_{b}_{qi}_{nh}")
                        nc.tensor.matmul(
                            yps[:],
                            onorm[:, qi * 128:(qi + 1) * 128],
                            wout_t[:, nh * 512:(nh + 1) * 512],
                            start=True, stop=True,
                        )
                        if nh == 0:
                            nc.scalar.copy(ysb[:, 0:512], yps[:])
                        else:
                            nc.vector.tensor_copy(ysb[:, 512:1024], yps[:])
                    nc.sync.dma_start(
                        out=y[b, qi * 128:(qi + 1) * 128, :], in_=ysb
                    )

            cur = proj_and_transpose(0, "b0")
            for b in range(B):
                onorm = onp.tile([128, SEQ], F32R, tag="onorm",
                                 name=f"onorm_{b}")
                stage_all = small.tile([2 * QC, 512], F32, tag="stage",
                                       name=f"stage_{b}")
                scr64 = small.tile([65, 512], F32, tag="scr64",
                                   name=f"scr64_{b}")
                attention(b, cur[0], cur[1], onorm, stage_all, scr64)
                if b + 1 < B:
                    cur = proj_and_transpose(b + 1, f"b{b + 1}")
                normalize(b, onorm, stage_all)
                wout_phase(b, onorm)

    return _patch_nc(nc)


def _causal_masks():
    m = np.zeros((4, 128, 512), np.float32)
    i = np.arange(128)[:, None]
    f = np.arange(512)[None, :]
    for j in range(4):
        m[j] = (f >= i + 128 * j).astype(np.float32)
    return m


def _prepare_in_maps(residual_stream, weight_query, weight_key, weight_value,
                     weight_out):
    xt = np.ascontiguousarray(
        np.asarray(residual_stream, np.float32).transpose(0, 2, 1)
    )
    masks = _causal_masks()
    ident = np.eye(128, dtype=np.float32)
    ones = np.ones((1, 64), np.float32)
    in_maps = []
    for c in range(NCORES):
        w = np.empty((128, MT, 3 * 128), np.float32)
        for proj, wt in ((0, weight_query), (1, weight_key), (2, weight_value)):
            # [1024, 128]: column h*64+d for core-local head h
            wc = np.asarray(wt[HPC * c:HPC * (c + 1)], np.float32)
            wc = wc.transpose(1, 0, 2).reshape(DM, HPC * DH)
            w[:, :, proj * 128:(proj + 1) * 128] = (
                wc.reshape(MT, 128, HPC * DH).transpose(1, 0, 2)
            )
        wo = np.ascontiguousarray(
            np.asarray(weight_out, np.float32)[128 * c:128 * (c + 1), :]
        )
        in_maps.append({
            "xt": xt,
            "w_all": np.ascontiguousarray(w),
            "wout": wo,
            "masks": masks,
            "ident": ident,
            "ones": ones,
        })
    return in_maps


def kernel(residual_stream, weight_query, weight_key, weight_value,
           weight_out, trace=False):
    from concourse.bass_utils import run_bass_kernel_spmd

    if "nc" not in _CACHE:
        _CACHE["nc"] = _build_nc()
    nc = _CACHE["nc"]

    in_maps = _prepare_in_maps(
        residual_stream, weight_query, weight_key, weight_value, weight_out
    )
    res = run_bass_kernel_spmd(
        nc, in_maps, list(range(NCORES)), trace=trace
    )
    _CACHE["last_result"] = res
    out = np.zeros((B, SEQ, DM), np.float32)
    for c in range(NCORES):
        out += res.results[c]["y"]
    return out


# revision 9
# speedup vs baseline: 1.2882x; 1.0707x over previous
"""Causal multi-head attention on 8 Trainium2 NeuronCores.

Problem: residual_stream [4, 2048, 1024] fp32, per-head QKV weights
[16, 1024, 64], output projection [1024, 1024]; causal softmax attention.

Sharding: tensor-parallel over heads — core c computes heads (2c, 2c+1)
for all 4 batches, applies the matching 128-row slice of weight_out, and
returns a full-shape partial output; the host sums the 8 partials
(all-reduce equivalent).

Per-core pipeline (all matmuls in f32r — full-rate fp32 mode):
  1. Q^T/K^T/V^T projections: stationary = weight m-tile, moving = X^T.
  2. V^T -> V via PE transpose; V augmented with a ones column so the
     P@V matmul also emits softmax row-sums for free.
  3. S^T = (Q K^T)^T blockwise, lower-triangle blocks only; the two
     heads are packed as concurrent K=64 row-group matmuls.
  4. P^T = exp(S^T/8) on ACT straight out of PSUM (scores are O(10), so
     no max-subtraction is needed); diagonal blocks get a 0/1 mask.
     PV runs two k-tiles behind S^T/exp (software pipeline).
  5. O_aug^T = V_aug^T P^T accumulated over k-tiles: rows 0:64 = O^T,
     row 64 = row-sums. Sums rows are staged to partitions 0..7, one
     batched reciprocal per batch, broadcast back via a DRAM bounce,
     then a single in-place multiply per (chunk, head).
  6. Y_partial = O_norm^T.T @ W_out[128c:128c+128] -> DRAM.

Phase order interleaves batch b's normalize/Wout tail behind batch
b+1's projections to keep the PE dense (HAM stays at full clock).
"""
import sys
import types

sys.path.insert(0, "/opt/trn_rl_repo")

import numpy as np

import concourse.bass as bass
import concourse.tile as tile
from concourse import mybir

F32 = mybir.dt.float32
F32R = mybir.dt.float32r

B = 4
SEQ = 2048
DM = 1024
DH = 64
NH = 16
NCORES = 8
HPC = NH // NCORES          # heads per core = 2
MT = DM // 128              # m-tiles = 8
KT = SEQ // 128             # k-tiles = 16
QC = SEQ // 512             # q-chunks of 512 = 4

_CACHE = {}


def _split_waits(d, max_waits=1):
    # This walrus build allows a single sync-wait on several instruction
    # encodings (CTRL Drain, fused-LDW f32 Matmult). Hoist excess waits
    # onto same-engine NoOp carriers directly in the BIR JSON.
    for fn in d.get("functions", []):
        for blk in fn.get("blocks", []):
            out = []
            for inst in blk.get("instructions", []):
                si = inst.get("sync_info") or {}
                waits = si.get("on_wait") or []
                if len(waits) > max_waits:
                    extra = waits[: len(waits) - max_waits]
                    rest = waits[len(waits) - max_waits:]
                    for i, w in enumerate(extra):
                        out.append({
                            "name": f"{inst['name']}_sw{i}",
                            "opcode": "NoOp",
                            "engine": inst["engine"],
                            "ins": [],
                            "outs": [],
                            "sync_info": {"on_update": [], "on_wait": [w]},
                        })
                    inst = dict(inst)
                    inst["sync_info"] = {
                        "on_update": list(si.get("on_update") or []),
                        "on_wait": rest,
                    }
                out.append(inst)
            blk["instructions"] = out
    return d


def _patch_nc(nc):
    import orjson

    def to_json_bytes(self):
        return orjson.dumps(
            _split_waits(orjson.loads(mybir.module_to_json_bytes(self.m)))
        )

    nc.to_json_bytes = types.MethodType(to_json_bytes, nc)
    return nc


def _build_nc():
    nc = bass.Bass()

    xt = nc.dram_tensor("xt", [B, DM, SEQ], F32R, kind="ExternalInput")
    w_all = nc.dram_tensor("w_all", [128, MT, 3 * 128], F32R, kind="ExternalInput")
    wout = nc.dram_tensor("wout", [128, DM], F32R, kind="ExternalInput")
    masks = nc.dram_tensor("masks", [4, 128, 512], F32, kind="ExternalInput")
    ident = nc.dram_tensor("ident", [128, 128], F32, kind="ExternalInput")
    ones = nc.dram_tensor("ones", [1, 64], F32R, kind="ExternalInput")
    y = nc.dram_tensor("y", [B, SEQ, DM], F32, kind="ExternalOutput")
    rscr = nc.dram_tensor("rscr", [B, 2 * QC, 512], F32)  # recip bounce

    with tile.TileContext(nc) as tc:
        with (
            tc.tile_pool(name="const", bufs=1) as const,
            tc.tile_pool(name="xtp", bufs=1) as xtp,
            tc.tile_pool(name="qkp", bufs=2) as qkp,
            tc.tile_pool(name="vtp", bufs=1) as vtp,
            tc.tile_pool(name="ptp", bufs=8) as ptp,
            tc.tile_pool(name="onp", bufs=1) as onp,
            tc.tile_pool(name="small", bufs=2) as small,
            tc.tile_pool(name="bcp", bufs=2) as bcp,
            tc.tile_pool(name="yp", bufs=2) as yp,
            tc.tile_pool(name="pss", bufs=4, space="PSUM") as pss,
            tc.tile_pool(name="pso", bufs=2, space="PSUM") as pso,
            tc.tile_pool(name="psf", bufs=2, space="PSUM") as psf,
        ):
            w_t = const.tile([128, MT, 3 * 128], F32R, tag="w")
            nc.sync.dma_start(out=w_t, in_=w_all[:])
            wout_t = const.tile([128, DM], F32R, tag="wout")
            nc.sync.dma_start(out=wout_t, in_=wout[:])
            mask_t = const.tile([128, 4, 512], F32, tag="mask")
            nc.sync.dma_start(
                out=mask_t, in_=masks[:].rearrange("j p f -> p j f")
            )
            ident_t = const.tile([128, 128], F32, tag="ident")
            nc.sync.dma_start(out=ident_t, in_=ident[:])

            # V_aug allocated once; ones column filled once.
            vaug = const.tile([128, KT, HPC, 65], F32R, tag="vaug")
            ones_src = bass.AP(
                tensor=ones[:].tensor,
                offset=ones[:].offset,
                ap=[[0, 128], [2, KT], [1, HPC], [1, 1]],
            )
            nc.gpsimd.dma_start(out=vaug[:, :, :, 64:65], in_=ones_src)

            def gen_proj(b, name, out):
                """QKV projections for batch b as PE thunks (QK pass, then
                V pass — at most 2 PSUM banks live)."""
                xt_t = xtp.tile([128, MT, SEQ], F32R, tag="xt",
                                name=f"xt_{name}")
                xv = xt[b].rearrange("(m p) q -> p m q", p=128)
                for m in range(MT):
                    nc.sync.dma_start(out=xt_t[:, m, :], in_=xv[:, m, :])
                qt = qkp.tile([128, SEQ], F32R, tag="qt", name=f"qt_{name}")
                kt = qkp.tile([128, SEQ], F32R, tag="kt", name=f"kt_{name}")
                vt = vtp.tile([128, SEQ], F32, tag="vt", name=f"vt_{name}")
                out["qt"], out["kt"], out["vt"] = qt, kt, vt
                thunks = []
                state = {}

                def qkstep(cq, m):
                    cqs = slice(cq * 512, cq * 512 + 512)
                    if m == 0:
                        state["pq"] = psf.tile([128, 512], F32, tag="bank",
                                               name=f"pq_{name}{cq}")
                        state["pk"] = psf.tile([128, 512], F32, tag="bank",
                                               name=f"pk_{name}{cq}")
                    for proj, key in ((0, "pq"), (1, "pk")):
                        nc.tensor.matmul(
                            state[key][:],
                            w_t[:, m, proj * 128:(proj + 1) * 128],
                            xt_t[:, m, cqs],
                            start=(m == 0),
                            stop=(m == MT - 1),
                        )
                    if m == MT - 1:
                        nc.scalar.copy(qt[:, cqs], state["pq"][:])
                        nc.scalar.copy(kt[:, cqs], state["pk"][:])

                def vstep(cq, m):
                    cqs = slice(cq * 512, cq * 512 + 512)
                    if m == 0:
                        state["pv"] = psf.tile([128, 512], F32, tag="bank",
                                               name=f"pv_{name}{cq}")
                    nc.tensor.matmul(
                        state["pv"][:],
                        w_t[:, m, 2 * 128:3 * 128],
                        xt_t[:, m, cqs],
                        start=(m == 0),
                        stop=(m == MT - 1),
                    )
                    if m == MT - 1:
                        nc.vector.tensor_copy(vt[:, cqs], state["pv"][:])

                for cq in range(QC):
                    for m in range(MT):
                        thunks.append(lambda cq=cq, m=m: qkstep(cq, m))
                for cq in range(QC):
                    for m in range(MT):
                        thunks.append(lambda cq=cq, m=m: vstep(cq, m))
                return thunks

            def gen_vtrans(name, vt):
                """V^T -> V_aug transposes as PE thunks."""
                def tstep(tk):
                    pt_ps = pso.tile([128, 128], F32, tag="bank",
                                     name=f"tp_{name}{tk}")
                    nc.tensor.transpose(
                        pt_ps[:], vt[:, tk * 128:(tk + 1) * 128], ident_t[:]
                    )
                    nc.vector.tensor_copy(vaug[:, tk, 0, 0:64], pt_ps[:, 0:64])
                    nc.vector.tensor_copy(vaug[:, tk, 1, 0:64],
                                          pt_ps[:, 64:128])
                return [lambda tk=tk: tstep(tk) for tk in range(KT)]

            def gen_attention(b, qt, kt, onorm, stage_all, scr64):
                """Attention thunks; one thunk per k-tile step."""
                thunks = []
                for cq in range(QC):
                    cqs = slice(cq * 512, cq * 512 + 512)
                    ntk = 4 * cq + 4
                    st = {"ops": None, "pend": []}

                    def make_pt(cq, tk, st):
                        cqs = slice(cq * 512, cq * 512 + 512)
                        sps = {}
                        for h in (0, 1):
                            hs = slice(h * 64, h * 64 + 64)
                            tks = slice(tk * 128, tk * 128 + 128)
                            spsum = pss.tile([128, 512], F32, tag="bank",
                                             name=f"s_b{b}c{cq}t{tk}h{h}")
                            nc.tensor.matmul(
                                spsum[:], kt[hs, tks], qt[hs, cqs],
                                start=True, stop=True,
                            )
                            sps[h] = spsum
                        pts = {}
                        for h in (0, 1):
                            pt = ptp.tile([128, 512], F32R, tag="pt",
                                          name=f"pt_b{b}c{cq}t{tk}h{h}")
                            nc.scalar.activation(
                                pt[:], sps[h][:],
                                mybir.ActivationFunctionType.Exp,
                                bias=0.0, scale=0.125,
                            )
                            if tk >= 4 * cq:
                                nc.vector.tensor_mul(
                                    pt[:], pt.bitcast(F32)[:],
                                    mask_t[:, tk - 4 * cq, :],
                                )
                            pts[h] = pt
                        return pts

                    def pv_step(cq, tk, pts, st, ntk):
                        for h in (0, 1):
                            nc.tensor.matmul(
                                st["ops"][h][:], vaug[:, tk, h, :],
                                pts[h][:],
                                start=(tk == 0), stop=(tk == ntk - 1),
                            )

                    def step(cq, tk, st, ntk):
                        if tk == 0:
                            st["ops"] = {
                                h: pso.tile([65, 512], F32, tag="bank",
                                            name=f"ops_b{b}c{cq}h{h}")
                                for h in (0, 1)
                            }
                        st["pend"].append((tk, make_pt(cq, tk, st)))
                        if len(st["pend"]) > 2:
                            t0, p0 = st["pend"].pop(0)
                            pv_step(cq, t0, p0, st, ntk)
                        if tk == ntk - 1:
                            while st["pend"]:
                                t0, p0 = st["pend"].pop(0)
                                pv_step(cq, t0, p0, st, ntk)
                            finish_chunk(cq, st)

                    def finish_chunk(cq, st):
                        cqs = slice(cq * 512, cq * 512 + 512)
                        for h in (0, 1):
                            p = 2 * cq + h
                            nc.scalar.copy(scr64[64:65, :],
                                           st["ops"][h][64:65, :])
                            nc.sync.dma_start(
                                out=stage_all[p:p + 1, :],
                                in_=scr64[64:65, :],
                            )
                            if h == 0:
                                nc.scalar.copy(
                                    onorm[0:64, cqs], st["ops"][h][0:64, :]
                                )
                            else:
                                nc.vector.tensor_copy(
                                    onorm[64:128, cqs], st["ops"][h][0:64, :]
                                )

                    for tk in range(ntk):
                        thunks.append(
                            lambda cq=cq, tk=tk, st=st, ntk=ntk:
                            step(cq, tk, st, ntk)
                        )
                return thunks

            def normalize(b, onorm, stage_all):
                recip = small.tile([2 * QC, 512], F32, tag="recip",
                                   name=f"recip_{b}")
                nc.vector.reciprocal(recip[:], stage_all[:])
                nc.sync.dma_start(out=rscr[b], in_=recip[:])
                for cq in range(QC):
                    cqs = slice(cq * 512, cq * 512 + 512)
                    for h in (0, 1):
                        p = 2 * cq + h
                        bc = bcp.tile([128, 512], F32, tag="bc",
                                      name=f"bc_b{b}c{cq}h{h}")
                        hs = slice(h * 64, h * 64 + 64)
                        bc_src = bass.AP(
                            tensor=rscr[:].tensor,
                            offset=(b * 2 * QC + p) * 512,
                            ap=[[0, 64], [1, 512]],
                        )
                        nc.gpsimd.dma_start(out=bc[hs, :], in_=bc_src)
                        nc.vector.tensor_mul(
                            onorm[hs, cqs], onorm.bitcast(F32)[hs, cqs],
                            bc[hs, :],
                        )

            def gen_wout(b, onorm):
                def wstep(qi):
                    ysb = yp.tile([128, DM], F32, tag="y", name=f"y_{b}_{qi}")
                    for nh in range(2):
                        yps = psf.tile([128, 512], F32, tag="bank",
                                       name=f"yps_{b}_{qi}_{nh}")
                        nc.tensor.matmul(
                            yps[:],
                            onorm[:, qi * 128:(qi + 1) * 128],
                            wout_t[:, nh * 512:(nh + 1) * 512],
                            start=True, stop=True,
                        )
                        if nh == 0:
                            nc.scalar.copy(ysb[:, 0:512], yps[:])
                        else:
                            nc.vector.tensor_copy(ysb[:, 512:1024], yps[:])
                    nc.sync.dma_start(
                        out=y[b, qi * 128:(qi + 1) * 128, :], in_=ysb
                    )
                return [lambda qi=qi: wstep(qi) for qi in range(KT)]

            def interleave(primary, fillers):
                """Run primary thunks, spreading filler thunks between them."""
                n, m = len(primary), len(fillers)
                fi = 0
                for i, t in enumerate(primary):
                    t()
                    want = (i + 1) * m // n
                    while fi < want:
                        fillers[fi]()
                        fi += 1
                while fi < m:
                    fillers[fi]()
                    fi += 1

            # Software-pipelined batch loop: batch b's attention interleaves
            # with batch b+1's projections and batch b-1's output matmuls,
            # keeping the PE dense through the ACT-bound attention phase.
            state = {}
            proj0 = gen_proj(0, "b0", state)
            for t in proj0:
                t()
            for t in gen_vtrans("b0", state["vt"]):
                t()
            prev_wout = []
            cur = state
            for b in range(B):
                onorm = onp.tile([128, SEQ], F32R, tag="onorm",
                                 name=f"onorm_{b}")
                stage_all = small.tile([2 * QC, 512], F32, tag="stage",
                                       name=f"stage_{b}")
                scr64 = small.tile([65, 512], F32, tag="scr64",
                                   name=f"scr64_{b}")
                attn = gen_attention(b, cur["qt"], cur["kt"], onorm,
                                     stage_all, scr64)
                fillers = list(prev_wout)
                nxt = {}
                if b + 1 < B:
                    fillers += gen_proj(b + 1, f"b{b + 1}", nxt)
                interleave(attn, fillers)
                if b + 1 < B:
                    for t in gen_vtrans(f"b{b + 1}", nxt["vt"]):
                        t()
                normalize(b, onorm, stage_all)
                prev_wout = gen_wout(b, onorm)
                cur = nxt
            for t in prev_wout:
                t()

    return _patch_nc(nc)


def _causal_masks():
    m = np.zeros((4, 128, 512), np.float32)
    i = np.arange(128)[:, None]
    f = np.arange(512)[None, :]
    for j in range(4):
        m[j] = (f >= i + 128 * j).astype(np.float32)
    return m


def _prepare_in_maps(residual_stream, weight_query, weight_key, weight_value,
                     weight_out):
    xt = np.ascontiguousarray(
        np.asarray(residual_stream, np.float32).transpose(0, 2, 1)
    )
    masks = _causal_masks()
    ident = np.eye(128, dtype=np.float32)
    ones = np.ones((1, 64), np.float32)
    in_maps = []
    for c in range(NCORES):
        w = np.empty((128, MT, 3 * 128), np.float32)
        for proj, wt in ((0, weight_query), (1, weight_key), (2, weight_value)):
            # [1024, 128]: column h*64+d for core-local head h
            wc = np.asarray(wt[HPC * c:HPC * (c + 1)], np.float32)
            wc = wc.transpose(1, 0, 2).reshape(DM, HPC * DH)
            w[:, :, proj * 128:(proj + 1) * 128] = (
                wc.reshape(MT, 128, HPC * DH).transpose(1, 0, 2)
            )
        wo = np.ascontiguousarray(
            np.asarray(weight_out, np.float32)[128 * c:128 * (c + 1), :]
        )
        in_maps.append({
            "xt": xt,
            "w_all": np.ascontiguousarray(w),
            "wout": wo,
            "masks": masks,
            "ident": ident,
            "ones": ones,
        })
    return in_maps


def kernel(residual_stream, weight_query, weight_key, weight_value,
           weight_out, trace=False):
    from concourse.bass_utils import run_bass_kernel_spmd

    if "nc" not in _CACHE:
        _CACHE["nc"] = _build_nc()
    nc = _CACHE["nc"]

    in_maps = _prepare_in_maps(
        residual_stream, weight_query, weight_key, weight_value, weight_out
    )
    res = run_bass_kernel_spmd(
        nc, in_maps, list(range(NCORES)), trace=trace
    )
    _CACHE["last_result"] = res
    out = np.zeros((B, SEQ, DM), np.float32)
    for c in range(NCORES):
        out += res.results[c]["y"]
    return out


# revision 10
# speedup vs baseline: 1.3179x; 1.0230x over previous
"""Causal multi-head attention on 8 Trainium2 NeuronCores.

Problem: residual_stream [4, 2048, 1024] fp32, per-head QKV weights
[16, 1024, 64], output projection [1024, 1024]; causal softmax attention.

Sharding: tensor-parallel over heads — core c computes heads (2c, 2c+1)
for all 4 batches, applies the matching 128-row slice of weight_out, and
returns a full-shape partial output; the host sums the 8 partials
(all-reduce equivalent).

Per-core pipeline (all matmuls in f32r — full-rate fp32 mode):
  1. Q^T/K^T/V^T projections: stationary = weight m-tile, moving = X^T.
  2. V^T -> V via PE transpose; V augmented with a ones column so the
     P@V matmul also emits softmax row-sums for free.
  3. S^T = (Q K^T)^T blockwise, lower-triangle blocks only; the two
     heads are packed as concurrent K=64 row-group matmuls.
  4. P^T = exp(S^T/8) on ACT straight out of PSUM (scores are O(10), so
     no max-subtraction is needed); diagonal blocks get a 0/1 mask.
     PV runs two k-tiles behind S^T/exp (software pipeline).
  5. O_aug^T = V_aug^T P^T accumulated over k-tiles: rows 0:64 = O^T,
     row 64 = row-sums. Sums rows are staged to partitions 0..7, one
     batched reciprocal per batch, broadcast back via a DRAM bounce,
     then a single in-place multiply per (chunk, head).
  6. Y_partial = O_norm^T.T @ W_out[128c:128c+128] -> DRAM.

Phase order interleaves batch b's normalize/Wout tail behind batch
b+1's projections to keep the PE dense (HAM stays at full clock).
"""
import sys
import types

sys.path.insert(0, "/opt/trn_rl_repo")

import numpy as np

import concourse.bass as bass
import concourse.tile as tile
from concourse import mybir

F32 = mybir.dt.float32
F32R = mybir.dt.float32r

B = 4
SEQ = 2048
DM = 1024
DH = 64
NH = 16
NCORES = 8
HPC = NH // NCORES          # heads per core = 2
MT = DM // 128              # m-tiles = 8
KT = SEQ // 128             # k-tiles = 16
QC = SEQ // 512             # q-chunks of 512 = 4

_CACHE = {}


def _split_waits(d, max_waits=1):
    # This walrus build allows a single sync-wait on several instruction
    # encodings (CTRL Drain, fused-LDW f32 Matmult). Hoist excess waits
    # onto same-engine NoOp carriers directly in the BIR JSON.
    for fn in d.get("functions", []):
        for blk in fn.get("blocks", []):
            out = []
            for inst in blk.get("instructions", []):
                si = inst.get("sync_info") or {}
                waits = si.get("on_wait") or []
                if len(waits) > max_waits:
                    extra = waits[: len(waits) - max_waits]
                    rest = waits[len(waits) - max_waits:]
                    for i, w in enumerate(extra):
                        out.append({
                            "name": f"{inst['name']}_sw{i}",
                            "opcode": "NoOp",
                            "engine": inst["engine"],
                            "ins": [],
                            "outs": [],
                            "sync_info": {"on_update": [], "on_wait": [w]},
                        })
                    inst = dict(inst)
                    inst["sync_info"] = {
                        "on_update": list(si.get("on_update") or []),
                        "on_wait": rest,
                    }
                out.append(inst)
            blk["instructions"] = out
    return d


def _patch_nc(nc):
    import orjson

    def to_json_bytes(self):
        return orjson.dumps(
            _split_waits(orjson.loads(mybir.module_to_json_bytes(self.m)))
        )

    nc.to_json_bytes = types.MethodType(to_json_bytes, nc)
    return nc


def _build_nc():
    nc = bass.Bass()

    xt = nc.dram_tensor("xt", [B, DM, SEQ], F32R, kind="ExternalInput")
    w_all = nc.dram_tensor("w_all", [128, MT, 3 * 128], F32R, kind="ExternalInput")
    wout = nc.dram_tensor("wout", [128, DM], F32R, kind="ExternalInput")
    masks = nc.dram_tensor("masks", [4, 128, 512], F32, kind="ExternalInput")
    ident = nc.dram_tensor("ident", [128, 128], F32, kind="ExternalInput")
    ones = nc.dram_tensor("ones", [1, 64], F32R, kind="ExternalInput")
    y = nc.dram_tensor("y", [B, SEQ, DM], F32, kind="ExternalOutput")
    rscr = nc.dram_tensor("rscr", [B, 2 * QC, 512], F32)  # recip bounce

    with tile.TileContext(nc) as tc:
        with (
            tc.tile_pool(name="const", bufs=1) as const,
            tc.tile_pool(name="xtp", bufs=5) as xtp,
            tc.tile_pool(name="qkp", bufs=2) as qkp,
            tc.tile_pool(name="vtp", bufs=1) as vtp,
            tc.tile_pool(name="ptp", bufs=6) as ptp,
            tc.tile_pool(name="onp", bufs=1) as onp,
            tc.tile_pool(name="small", bufs=2) as small,
            tc.tile_pool(name="bcp", bufs=2) as bcp,
            tc.tile_pool(name="yp", bufs=2) as yp,
            tc.tile_pool(name="pss", bufs=4, space="PSUM") as pss,
            tc.tile_pool(name="pso", bufs=2, space="PSUM") as pso,
            tc.tile_pool(name="psf", bufs=2, space="PSUM") as psf,
        ):
            w_t = const.tile([128, MT, 3 * 128], F32R, tag="w")
            nc.sync.dma_start(out=w_t, in_=w_all[:])
            wout_t = const.tile([128, DM], F32R, tag="wout")
            nc.sync.dma_start(out=wout_t, in_=wout[:])
            mask_t = const.tile([128, 4, 512], F32, tag="mask")
            nc.sync.dma_start(
                out=mask_t, in_=masks[:].rearrange("j p f -> p j f")
            )
            ident_t = const.tile([128, 128], F32, tag="ident")
            nc.sync.dma_start(out=ident_t, in_=ident[:])

            # V_aug allocated once; ones column filled once.
            vaug = const.tile([128, KT, HPC, 65], F32R, tag="vaug")
            ones_src = bass.AP(
                tensor=ones[:].tensor,
                offset=ones[:].offset,
                ap=[[0, 128], [2, KT], [1, HPC], [1, 1]],
            )
            nc.gpsimd.dma_start(out=vaug[:, :, :, 64:65], in_=ones_src)

            def gen_proj(b, name, out):
                """QKV projections for batch b as PE thunks. X^T is loaded
                and released at (q-chunk, m-tile) granularity so the next
                batch's prefetch overlaps this batch's attention."""
                xv = xt[b].rearrange("(m p) q -> p m q", p=128)
                xts = []
                for cq in range(QC):
                    xt_t = xtp.tile([128, MT, 512], F32R, tag="xt",
                                    name=f"xt_{name}c{cq}")
                    for m in range(MT):
                        nc.sync.dma_start(
                            out=xt_t[:, m, :],
                            in_=xv[:, m, cq * 512:(cq + 1) * 512],
                        )
                    xts.append(xt_t)
                qt = qkp.tile([128, SEQ], F32R, tag="qt", name=f"qt_{name}")
                kt = qkp.tile([128, SEQ], F32R, tag="kt", name=f"kt_{name}")
                vt = vtp.tile([128, SEQ], F32, tag="vt", name=f"vt_{name}")
                out["qt"], out["kt"], out["vt"] = qt, kt, vt
                thunks = []
                state = {}

                def qkstep(cq, m):
                    cqs = slice(cq * 512, cq * 512 + 512)
                    if m == 0:
                        state["pq"] = psf.tile([128, 512], F32, tag="bank",
                                               name=f"pq_{name}{cq}")
                        state["pk"] = psf.tile([128, 512], F32, tag="bank",
                                               name=f"pk_{name}{cq}")
                    for proj, key in ((0, "pq"), (1, "pk")):
                        nc.tensor.matmul(
                            state[key][:],
                            w_t[:, m, proj * 128:(proj + 1) * 128],
                            xts[cq][:, m, :],
                            start=(m == 0),
                            stop=(m == MT - 1),
                        )
                    if m == MT - 1:
                        nc.scalar.copy(qt[:, cqs], state["pq"][:])
                        nc.scalar.copy(kt[:, cqs], state["pk"][:])

                def vstep(cq, m):
                    cqs = slice(cq * 512, cq * 512 + 512)
                    if m == 0:
                        state["pv"] = psf.tile([128, 512], F32, tag="bank",
                                               name=f"pv_{name}{cq}")
                    nc.tensor.matmul(
                        state["pv"][:],
                        w_t[:, m, 2 * 128:3 * 128],
                        xts[cq][:, m, :],
                        start=(m == 0),
                        stop=(m == MT - 1),
                    )
                    if m == MT - 1:
                        nc.vector.tensor_copy(vt[:, cqs], state["pv"][:])

                for cq in range(QC):
                    for m in range(MT):
                        thunks.append(lambda cq=cq, m=m: qkstep(cq, m))
                    for m in range(MT):
                        thunks.append(lambda cq=cq, m=m: vstep(cq, m))
                return thunks

            def gen_vtrans(name, vt):
                """V^T -> V_aug transposes as PE thunks."""
                def tstep(tk):
                    pt_ps = pso.tile([128, 128], F32, tag="bank",
                                     name=f"tp_{name}{tk}")
                    nc.tensor.transpose(
                        pt_ps[:], vt[:, tk * 128:(tk + 1) * 128], ident_t[:]
                    )
                    nc.vector.tensor_copy(vaug[:, tk, 0, 0:64], pt_ps[:, 0:64])
                    nc.vector.tensor_copy(vaug[:, tk, 1, 0:64],
                                          pt_ps[:, 64:128])
                return [lambda tk=tk: tstep(tk) for tk in range(KT)]

            def gen_attention(b, qt, kt, onorm, stage_all, scr64):
                """Attention thunks; one thunk per k-tile step."""
                thunks = []
                for cq in range(QC):
                    cqs = slice(cq * 512, cq * 512 + 512)
                    ntk = 4 * cq + 4
                    st = {"ops": None, "pend": []}

                    def make_pt(cq, tk, st):
                        cqs = slice(cq * 512, cq * 512 + 512)
                        sps = {}
                        for h in (0, 1):
                            hs = slice(h * 64, h * 64 + 64)
                            tks = slice(tk * 128, tk * 128 + 128)
                            spsum = pss.tile([128, 512], F32, tag="bank",
                                             name=f"s_b{b}c{cq}t{tk}h{h}")
                            nc.tensor.matmul(
                                spsum[:], kt[hs, tks], qt[hs, cqs],
                                start=True, stop=True,
                            )
                            sps[h] = spsum
                        pts = {}
                        for h in (0, 1):
                            pt = ptp.tile([128, 512], F32R, tag="pt",
                                          name=f"pt_b{b}c{cq}t{tk}h{h}")
                            nc.scalar.activation(
                                pt[:], sps[h][:],
                                mybir.ActivationFunctionType.Exp,
                                bias=0.0, scale=0.125,
                            )
                            if tk >= 4 * cq:
                                nc.vector.tensor_mul(
                                    pt[:], pt.bitcast(F32)[:],
                                    mask_t[:, tk - 4 * cq, :],
                                )
                            pts[h] = pt
                        return pts

                    def pv_step(cq, tk, pts, st, ntk):
                        for h in (0, 1):
                            nc.tensor.matmul(
                                st["ops"][h][:], vaug[:, tk, h, :],
                                pts[h][:],
                                start=(tk == 0), stop=(tk == ntk - 1),
                            )

                    def step(cq, tk, st, ntk):
                        if tk == 0:
                            st["ops"] = {
                                h: pso.tile([65, 512], F32, tag="bank",
                                            name=f"ops_b{b}c{cq}h{h}")
                                for h in (0, 1)
                            }
                        st["pend"].append((tk, make_pt(cq, tk, st)))
                        if len(st["pend"]) > 2:
                            t0, p0 = st["pend"].pop(0)
                            pv_step(cq, t0, p0, st, ntk)
                        if tk == ntk - 1:
                            while st["pend"]:
                                t0, p0 = st["pend"].pop(0)
                                pv_step(cq, t0, p0, st, ntk)
                            finish_chunk(cq, st)

                    def finish_chunk(cq, st):
                        cqs = slice(cq * 512, cq * 512 + 512)
                        for h in (0, 1):
                            p = 2 * cq + h
                            nc.scalar.copy(scr64[64:65, :],
                                           st["ops"][h][64:65, :])
                            nc.sync.dma_start(
                                out=stage_all[p:p + 1, :],
                                in_=scr64[64:65, :],
                            )
                            if h == 0:
                                nc.scalar.copy(
                                    onorm[0:64, cqs], st["ops"][h][0:64, :]
                                )
                            else:
                                nc.vector.tensor_copy(
                                    onorm[64:128, cqs], st["ops"][h][0:64, :]
                                )

                    for tk in range(ntk):
                        thunks.append(
                            lambda cq=cq, tk=tk, st=st, ntk=ntk:
                            step(cq, tk, st, ntk)
                        )
                return thunks

            def normalize(b, onorm, stage_all):
                recip = small.tile([2 * QC, 512], F32, tag="recip",
                                   name=f"recip_{b}")
                nc.vector.reciprocal(recip[:], stage_all[:])
                nc.sync.dma_start(out=rscr[b], in_=recip[:])
                for cq in range(QC):
                    cqs = slice(cq * 512, cq * 512 + 512)
                    for h in (0, 1):
                        p = 2 * cq + h
                        bc = bcp.tile([128, 512], F32, tag="bc",
                                      name=f"bc_b{b}c{cq}h{h}")
                        hs = slice(h * 64, h * 64 + 64)
                        bc_src = bass.AP(
                            tensor=rscr[:].tensor,
                            offset=(b * 2 * QC + p) * 512,
                            ap=[[0, 64], [1, 512]],
                        )
                        nc.gpsimd.dma_start(out=bc[hs, :], in_=bc_src)
                        nc.vector.tensor_mul(
                            onorm[hs, cqs], onorm.bitcast(F32)[hs, cqs],
                            bc[hs, :],
                        )

            def gen_wout(b, onorm):
                def wstep(qi):
                    ysb = yp.tile([128, DM], F32, tag="y", name=f"y_{b}_{qi}")
                    for nh in range(2):
                        yps = psf.tile([128, 512], F32, tag="bank",
                                       name=f"yps_{b}_{qi}_{nh}")
                        nc.tensor.matmul(
                            yps[:],
                            onorm[:, qi * 128:(qi + 1) * 128],
                            wout_t[:, nh * 512:(nh + 1) * 512],
                            start=True, stop=True,
                        )
                        if nh == 0:
                            nc.scalar.copy(ysb[:, 0:512], yps[:])
                        else:
                            nc.vector.tensor_copy(ysb[:, 512:1024], yps[:])
                    nc.sync.dma_start(
                        out=y[b, qi * 128:(qi + 1) * 128, :], in_=ysb
                    )
                return [lambda qi=qi: wstep(qi) for qi in range(KT)]

            def interleave(primary, fillers):
                """Run primary thunks, spreading filler thunks between them."""
                n, m = len(primary), len(fillers)
                fi = 0
                for i, t in enumerate(primary):
                    t()
                    want = (i + 1) * m // n
                    while fi < want:
                        fillers[fi]()
                        fi += 1
                while fi < m:
                    fillers[fi]()
                    fi += 1

            # Software-pipelined batch loop: batch b's attention interleaves
            # with batch b+1's projections and batch b-1's output matmuls,
            # keeping the PE dense through the ACT-bound attention phase.
            state = {}
            proj0 = gen_proj(0, "b0", state)
            for t in proj0:
                t()
            for t in gen_vtrans("b0", state["vt"]):
                t()
            prev_wout = []
            cur = state
            for b in range(B):
                onorm = onp.tile([128, SEQ], F32R, tag="onorm",
                                 name=f"onorm_{b}")
                stage_all = small.tile([2 * QC, 512], F32, tag="stage",
                                       name=f"stage_{b}")
                scr64 = small.tile([65, 512], F32, tag="scr64",
                                   name=f"scr64_{b}")
                attn = gen_attention(b, cur["qt"], cur["kt"], onorm,
                                     stage_all, scr64)
                fillers = list(prev_wout)
                nxt = {}
                if b + 1 < B:
                    fillers += gen_proj(b + 1, f"b{b + 1}", nxt)
                interleave(attn, fillers)
                if b + 1 < B:
                    for t in gen_vtrans(f"b{b + 1}", nxt["vt"]):
                        t()
                normalize(b, onorm, stage_all)
                prev_wout = gen_wout(b, onorm)
                cur = nxt
            for t in prev_wout:
                t()

    return _patch_nc(nc)


def _causal_masks():
    m = np.zeros((4, 128, 512), np.float32)
    i = np.arange(128)[:, None]
    f = np.arange(512)[None, :]
    for j in range(4):
        m[j] = (f >= i + 128 * j).astype(np.float32)
    return m


def _prepare_in_maps(residual_stream, weight_query, weight_key, weight_value,
                     weight_out):
    xt = np.ascontiguousarray(
        np.asarray(residual_stream, np.float32).transpose(0, 2, 1)
    )
    masks = _causal_masks()
    ident = np.eye(128, dtype=np.float32)
    ones = np.ones((1, 64), np.float32)
    in_maps = []
    for c in range(NCORES):
        w = np.empty((128, MT, 3 * 128), np.float32)
        for proj, wt in ((0, weight_query), (1, weight_key), (2, weight_value)):
            # [1024, 128]: column h*64+d for core-local head h
            wc = np.asarray(wt[HPC * c:HPC * (c + 1)], np.float32)
            wc = wc.transpose(1, 0, 2).reshape(DM, HPC * DH)
            w[:, :, proj * 128:(proj + 1) * 128] = (
                wc.reshape(MT, 128, HPC * DH).transpose(1, 0, 2)
            )
        wo = np.ascontiguousarray(
            np.asarray(weight_out, np.float32)[128 * c:128 * (c + 1), :]
        )
        in_maps.append({
            "xt": xt,
            "w_all": np.ascontiguousarray(w),
            "wout": wo,
            "masks": masks,
            "ident": ident,
            "ones": ones,
        })
    return in_maps


def kernel(residual_stream, weight_query, weight_key, weight_value,
           weight_out, trace=False):
    from concourse.bass_utils import run_bass_kernel_spmd

    if "nc" not in _CACHE:
        _CACHE["nc"] = _build_nc()
    nc = _CACHE["nc"]

    in_maps = _prepare_in_maps(
        residual_stream, weight_query, weight_key, weight_value, weight_out
    )
    res = run_bass_kernel_spmd(
        nc, in_maps, list(range(NCORES)), trace=trace
    )
    _CACHE["last_result"] = res
    out = np.zeros((B, SEQ, DM), np.float32)
    for c in range(NCORES):
        out += res.results[c]["y"]
    return out


# revision 14
# speedup vs baseline: 1.6256x; 1.2335x over previous
"""Causal multi-head attention on 8 Trainium2 NeuronCores.

Problem: residual_stream [4, 2048, 1024] fp32, per-head QKV weights
[16, 1024, 64], output projection [1024, 1024]; causal softmax attention.

Sharding: tensor-parallel over heads — core c computes heads (2c, 2c+1)
for all 4 batches, applies the matching 128-row slice of weight_out, and
returns a full-shape partial output; the host sums the 8 partials
(all-reduce equivalent).

Per-core pipeline (all matmuls in f32r — full-rate fp32 mode):
  1. Q^T/K^T/V^T projections: stationary = weight m-tile, moving = X^T.
  2. V^T -> V via PE transpose; V augmented with a ones column so the
     P@V matmul also emits softmax row-sums for free.
  3. S^T = (Q K^T)^T blockwise, lower-triangle blocks only; the two
     heads are packed as concurrent K=64 row-group matmuls.
  4. P^T = exp(S^T/8) on ACT straight out of PSUM (scores are O(10), so
     no max-subtraction is needed); diagonal blocks get a 0/1 mask.
     PV runs two k-tiles behind S^T/exp (software pipeline).
  5. O_aug^T = V_aug^T P^T accumulated over k-tiles: rows 0:64 = O^T,
     row 64 = row-sums. Sums rows are staged to partitions 0..7, one
     batched reciprocal per batch, broadcast back via a DRAM bounce,
     then a single in-place multiply per (chunk, head).
  6. Y_partial = O_norm^T.T @ W_out[128c:128c+128] -> DRAM.

Phase order interleaves batch b's normalize/Wout tail behind batch
b+1's projections to keep the PE dense (HAM stays at full clock).
"""
import sys
import types

sys.path.insert(0, "/opt/trn_rl_repo")

import numpy as np

import concourse.bass as bass
import concourse.tile as tile
from concourse import mybir

F32 = mybir.dt.float32
F32R = mybir.dt.float32r

B = 4
SEQ = 2048
DM = 1024
DH = 64
NH = 16
NCORES = 8
HPC = NH // NCORES          # heads per core = 2
MT = DM // 128              # m-tiles = 8
KT = SEQ // 128             # k-tiles = 16
QC = SEQ // 512             # q-chunks of 512 = 4

_CACHE = {}


def _split_waits(d, max_waits=1):
    # This walrus build allows a single sync-wait on several instruction
    # encodings (CTRL Drain, fused-LDW f32 Matmult). Hoist excess waits
    # onto same-engine NoOp carriers directly in the BIR JSON.
    for fn in d.get("functions", []):
        for blk in fn.get("blocks", []):
            out = []
            for inst in blk.get("instructions", []):
                si = inst.get("sync_info") or {}
                waits = si.get("on_wait") or []
                if len(waits) > max_waits:
                    extra = waits[: len(waits) - max_waits]
                    rest = waits[len(waits) - max_waits:]
                    for i, w in enumerate(extra):
                        out.append({
                            "name": f"{inst['name']}_sw{i}",
                            "opcode": "NoOp",
                            "engine": inst["engine"],
                            "ins": [],
                            "outs": [],
                            "sync_info": {"on_update": [], "on_wait": [w]},
                        })
                    inst = dict(inst)
                    inst["sync_info"] = {
                        "on_update": list(si.get("on_update") or []),
                        "on_wait": rest,
                    }
                out.append(inst)
            blk["instructions"] = out
    return d


def _patch_nc(nc):
    import orjson

    def to_json_bytes(self):
        return orjson.dumps(
            _split_waits(orjson.loads(mybir.module_to_json_bytes(self.m)))
        )

    nc.to_json_bytes = types.MethodType(to_json_bytes, nc)
    return nc


def _build_nc():
    nc = bass.Bass()

    xt = nc.dram_tensor("xt", [B, DM, SEQ], F32R, kind="ExternalInput")
    w_all = nc.dram_tensor("w_all", [128, MT, 3 * 128], F32R, kind="ExternalInput")
    wout = nc.dram_tensor("wout", [128, DM], F32R, kind="ExternalInput")
    masks = nc.dram_tensor("masks", [4, 128, 512], F32, kind="ExternalInput")
    ident = nc.dram_tensor("ident", [128, 128], F32, kind="ExternalInput")
    ones = nc.dram_tensor("ones", [1, 64], F32R, kind="ExternalInput")
    y = nc.dram_tensor("y", [B, SEQ, DM], F32, kind="ExternalOutput")
    rscr = nc.dram_tensor("rscr", [B, 2 * QC, 512], F32)  # recip bounce

    with tile.TileContext(nc) as tc:
        with (
            tc.tile_pool(name="const", bufs=1) as const,
            tc.tile_pool(name="xtp", bufs=4) as xtp,
            tc.tile_pool(name="qkp", bufs=2) as qkp,
            tc.tile_pool(name="vtp", bufs=1) as vtp,
            tc.tile_pool(name="ptp", bufs=4) as ptp,
            tc.tile_pool(name="onp", bufs=2) as onp,
            tc.tile_pool(name="small", bufs=1) as small,
            tc.tile_pool(name="bcp", bufs=2) as bcp,
            tc.tile_pool(name="yp", bufs=2) as yp,
            tc.tile_pool(name="pss", bufs=2, space="PSUM") as pss,
            tc.tile_pool(name="pso", bufs=2, space="PSUM") as pso,
            tc.tile_pool(name="psf", bufs=2, space="PSUM") as psf,
        ):
            w_t = const.tile([128, MT, 3 * 128], F32R, tag="w")
            nc.sync.dma_start(out=w_t, in_=w_all[:])
            wout_t = const.tile([128, DM], F32R, tag="wout")
            nc.sync.dma_start(out=wout_t, in_=wout[:])
            mask_t = const.tile([128, 4, 512], F32, tag="mask")
            nc.sync.dma_start(
                out=mask_t, in_=masks[:].rearrange("j p f -> p j f")
            )
            ident_t = const.tile([128, 128], F32, tag="ident")
            nc.sync.dma_start(out=ident_t, in_=ident[:])

            # V_aug allocated once; ones column filled once.
            vaug = const.tile([128, KT, HPC, 65], F32R, tag="vaug")
            ones_src = bass.AP(
                tensor=ones[:].tensor,
                offset=ones[:].offset,
                ap=[[0, 128], [2, KT], [1, HPC], [1, 1]],
            )
            nc.gpsimd.dma_start(out=vaug[:, :, :, 64:65], in_=ones_src)

            def gen_proj(b, name, out):
                """QKV projections for batch b as PE thunks. X^T is loaded
                and released at (q-chunk, m-tile) granularity so the next
                batch's prefetch overlaps this batch's attention."""
                xv = xt[b].rearrange("(m p) q -> p m q", p=128)
                xts = []
                for cq in range(QC):
                    xt_t = xtp.tile([128, MT, 512], F32R, tag="xt",
                                    name=f"xt_{name}c{cq}")
                    for m in range(MT):
                        nc.sync.dma_start(
                            out=xt_t[:, m, :],
                            in_=xv[:, m, cq * 512:(cq + 1) * 512],
                        )
                    xts.append(xt_t)
                qt = qkp.tile([128, SEQ], F32R, tag="qt", name=f"qt_{name}")
                kt = qkp.tile([128, SEQ], F32R, tag="kt", name=f"kt_{name}")
                vt = vtp.tile([128, SEQ], F32, tag="vt", name=f"vt_{name}")
                out["qt"], out["kt"], out["vt"] = qt, kt, vt
                thunks = []
                state = {}

                def qkstep(cq, m):
                    cqs = slice(cq * 512, cq * 512 + 512)
                    if m == 0:
                        state["pq"] = psf.tile([128, 512], F32, tag="bank",
                                               name=f"pq_{name}{cq}")
                        state["pk"] = psf.tile([128, 512], F32, tag="bank",
                                               name=f"pk_{name}{cq}")
                    for proj, key in ((0, "pq"), (1, "pk")):
                        nc.tensor.matmul(
                            state[key][:],
                            w_t[:, m, proj * 128:(proj + 1) * 128],
                            xts[cq][:, m, :],
                            start=(m == 0),
                            stop=(m == MT - 1),
                        )
                    if m == MT - 1:
                        nc.scalar.copy(qt[:, cqs], state["pq"][:])
                        nc.scalar.copy(kt[:, cqs], state["pk"][:])

                def vstep(cq, m):
                    cqs = slice(cq * 512, cq * 512 + 512)
                    if m == 0:
                        state["pv"] = psf.tile([128, 512], F32, tag="bank",
                                               name=f"pv_{name}{cq}")
                    nc.tensor.matmul(
                        state["pv"][:],
                        w_t[:, m, 2 * 128:3 * 128],
                        xts[cq][:, m, :],
                        start=(m == 0),
                        stop=(m == MT - 1),
                    )
                    if m == MT - 1:
                        nc.vector.tensor_copy(vt[:, cqs], state["pv"][:])

                for cq in range(QC):
                    for m in range(MT):
                        thunks.append(lambda cq=cq, m=m: qkstep(cq, m))
                    for m in range(MT):
                        thunks.append(lambda cq=cq, m=m: vstep(cq, m))
                return thunks

            def gen_vtrans(name, vt):
                """V^T -> V_aug transposes as PE thunks."""
                def tstep(tk):
                    pt_ps = psf.tile([128, 128], F32, tag="bank",
                                     name=f"tp_{name}{tk}")
                    nc.tensor.transpose(
                        pt_ps[:], vt[:, tk * 128:(tk + 1) * 128], ident_t[:]
                    )
                    nc.vector.tensor_copy(vaug[:, tk, 0, 0:64], pt_ps[:, 0:64])
                    nc.vector.tensor_copy(vaug[:, tk, 1, 0:64],
                                          pt_ps[:, 64:128])
                return [lambda tk=tk: tstep(tk) for tk in range(KT)]

            def gen_attention(b, qt, kt, onorm, stage_all, scr64):
                """Attention thunks; one thunk per k-tile step. The two
                heads' S^T blocks land in one [128,1024] PSUM pair and are
                exponentiated by a single ACT op."""
                thunks = []
                for cq in range(QC):
                    ntk = 4 * cq + 4
                    st = {"ops": None, "pend": []}

                    def make_pt(cq, tk):
                        cqs = slice(cq * 512, cq * 512 + 512)
                        tks = slice(tk * 128, tk * 128 + 128)
                        sdbl = pss.tile([128, 1024], F32, tag="bank",
                                        name=f"s_b{b}c{cq}t{tk}")
                        for h in (0, 1):
                            hs = slice(h * 64, h * 64 + 64)
                            nc.tensor.matmul(
                                sdbl[:, h * 512:(h + 1) * 512],
                                kt[hs, tks], qt[hs, cqs],
                                start=True, stop=True,
                            )
                        pt = ptp.tile([128, 1024], F32R, tag="pt",
                                      name=f"pt_b{b}c{cq}t{tk}")
                        nc.scalar.activation(
                            pt[:], sdbl[:],
                            mybir.ActivationFunctionType.Exp,
                            bias=0.0, scale=0.125,
                        )
                        if tk >= 4 * cq:
                            for h in (0, 1):
                                nc.vector.tensor_mul(
                                    pt[:, h * 512:(h + 1) * 512],
                                    pt.bitcast(F32)[:, h * 512:(h + 1) * 512],
                                    mask_t[:, tk - 4 * cq, :],
                                )
                        return pt

                    def pv_step(cq, tk, pt, st, ntk):
                        for h in (0, 1):
                            nc.tensor.matmul(
                                st["ops"][h][:], vaug[:, tk, h, :],
                                pt[:, h * 512:(h + 1) * 512],
                                start=(tk == 0), stop=(tk == ntk - 1),
                            )

                    def step(cq, tk, st, ntk):
                        if tk == 0:
                            st["ops"] = {
                                h: pso.tile([65, 512], F32, tag="bank",
                                            name=f"ops_b{b}c{cq}h{h}")
                                for h in (0, 1)
                            }
                        st["pend"].append((tk, make_pt(cq, tk)))
                        if len(st["pend"]) > 2:
                            t0, p0 = st["pend"].pop(0)
                            pv_step(cq, t0, p0, st, ntk)
                        if tk == ntk - 1:
                            while st["pend"]:
                                t0, p0 = st["pend"].pop(0)
                                pv_step(cq, t0, p0, st, ntk)
                            finish_chunk(cq, st)

                    def finish_chunk(cq, st):
                        cqs = slice(cq * 512, cq * 512 + 512)
                        for h in (0, 1):
                            p = 2 * cq + h
                            nc.scalar.copy(scr64[64:65, :],
                                           st["ops"][h][64:65, :])
                            nc.sync.dma_start(
                                out=stage_all[p:p + 1, :],
                                in_=scr64[64:65, :],
                            )
                            if h == 0:
                                nc.scalar.copy(
                                    onorm[0:64, cqs], st["ops"][h][0:64, :]
                                )
                            else:
                                nc.vector.tensor_copy(
                                    onorm[64:128, cqs], st["ops"][h][0:64, :]
                                )

                    for tk in range(ntk):
                        thunks.append(
                            lambda cq=cq, tk=tk, st=st, ntk=ntk:
                            step(cq, tk, st, ntk)
                        )
                return thunks

            def normalize(b, onorm, stage_all):
                recip = small.tile([2 * QC, 512], F32, tag="recip",
                                   name=f"recip_{b}")
                nc.vector.reciprocal(recip[:], stage_all[:])
                nc.sync.dma_start(out=rscr[b], in_=recip[:])
                for cq in range(QC):
                    cqs = slice(cq * 512, cq * 512 + 512)
                    for h in (0, 1):
                        p = 2 * cq + h
                        bc = bcp.tile([128, 512], F32, tag="bc",
                                      name=f"bc_b{b}c{cq}h{h}")
                        hs = slice(h * 64, h * 64 + 64)
                        bc_src = bass.AP(
                            tensor=rscr[:].tensor,
                            offset=(b * 2 * QC + p) * 512,
                            ap=[[0, 64], [1, 512]],
                        )
                        nc.gpsimd.dma_start(out=bc[hs, :], in_=bc_src)
                        nc.vector.tensor_mul(
                            onorm[hs, cqs], onorm.bitcast(F32)[hs, cqs],
                            bc[hs, :],
                        )

            def gen_wout(b, onorm):
                def wstep(qi):
                    ysb = yp.tile([128, DM], F32, tag="y", name=f"y_{b}_{qi}")
                    for nh in range(2):
                        yps = psf.tile([128, 512], F32, tag="bank",
                                       name=f"yps_{b}_{qi}_{nh}")
                        nc.tensor.matmul(
                            yps[:],
                            onorm[:, qi * 128:(qi + 1) * 128],
                            wout_t[:, nh * 512:(nh + 1) * 512],
                            start=True, stop=True,
                        )
                        if nh == 0:
                            nc.scalar.copy(ysb[:, 0:512], yps[:])
                        else:
                            nc.vector.tensor_copy(ysb[:, 512:1024], yps[:])
                    nc.sync.dma_start(
                        out=y[b, qi * 128:(qi + 1) * 128, :], in_=ysb
                    )
                return [lambda qi=qi: wstep(qi) for qi in range(KT)]

            def interleave(primary, fillers):
                """Run primary thunks, spreading filler thunks between them."""
                n, m = len(primary), len(fillers)
                fi = 0
                for i, t in enumerate(primary):
                    t()
                    want = (i + 1) * m // n
                    while fi < want:
                        fillers[fi]()
                        fi += 1
                while fi < m:
                    fillers[fi]()
                    fi += 1

            # Software-pipelined batch loop: batch b's attention interleaves
            # with batch b+1's projections and batch b-1's output matmuls,
            # keeping the PE dense through the ACT-bound attention phase.
            state = {}
            proj0 = gen_proj(0, "b0", state)
            for t in proj0:
                t()
            for t in gen_vtrans("b0", state["vt"]):
                t()
            prev_wout = []
            cur = state
            for b in range(B):
                onorm = onp.tile([128, SEQ], F32R, tag="onorm",
                                 name=f"onorm_{b}")
                stage_all = small.tile([2 * QC, 512], F32, tag="stage",
                                       name=f"stage_{b}")
                scr64 = small.tile([65, 512], F32, tag="scr64",
                                   name=f"scr64_{b}")
                attn = gen_attention(b, cur["qt"], cur["kt"], onorm,
                                     stage_all, scr64)
                fillers = []
                nxt = {}
                if b + 1 < B:
                    fillers += gen_proj(b + 1, f"b{b + 1}", nxt)
                fillers += prev_wout
                interleave(attn, fillers)
                # vaug is a single shared tile: its writes must stay
                # structurally after all of attention(b)'s PV reads.
                if b + 1 < B:
                    for t in gen_vtrans(f"b{b + 1}", nxt["vt"]):
                        t()
                normalize(b, onorm, stage_all)
                prev_wout = gen_wout(b, onorm)
                cur = nxt
            for t in prev_wout:
                t()

    return _patch_nc(nc)


def _causal_masks():
    m = np.zeros((4, 128, 512), np.float32)
    i = np.arange(128)[:, None]
    f = np.arange(512)[None, :]
    for j in range(4):
        m[j] = (f >= i + 128 * j).astype(np.float32)
    return m


def _prepare_in_maps(residual_stream, weight_query, weight_key, weight_value,
                     weight_out):
    xt = np.ascontiguousarray(
        np.asarray(residual_stream, np.float32).transpose(0, 2, 1)
    )
    masks = _causal_masks()
    ident = np.eye(128, dtype=np.float32)
    ones = np.ones((1, 64), np.float32)
    in_maps = []
    for c in range(NCORES):
        w = np.empty((128, MT, 3 * 128), np.float32)
        for proj, wt in ((0, weight_query), (1, weight_key), (2, weight_value)):
            # [1024, 128]: column h*64+d for core-local head h
            wc = np.asarray(wt[HPC * c:HPC * (c + 1)], np.float32)
            wc = wc.transpose(1, 0, 2).reshape(DM, HPC * DH)
            w[:, :, proj * 128:(proj + 1) * 128] = (
                wc.reshape(MT, 128, HPC * DH).transpose(1, 0, 2)
            )
        wo = np.ascontiguousarray(
            np.asarray(weight_out, np.float32)[128 * c:128 * (c + 1), :]
        )
        in_maps.append({
            "xt": xt,
            "w_all": np.ascontiguousarray(w),
            "wout": wo,
            "masks": masks,
            "ident": ident,
            "ones": ones,
        })
    return in_maps


def kernel(residual_stream, weight_query, weight_key, weight_value,
           weight_out, trace=False):
    from concourse.bass_utils import run_bass_kernel_spmd

    if "nc" not in _CACHE:
        _CACHE["nc"] = _build_nc()
    nc = _CACHE["nc"]

    in_maps = _prepare_in_maps(
        residual_stream, weight_query, weight_key, weight_value, weight_out
    )
    res = run_bass_kernel_spmd(
        nc, in_maps, list(range(NCORES)), trace=trace
    )
    _CACHE["last_result"] = res
    out = np.zeros((B, SEQ, DM), np.float32)
    for c in range(NCORES):
        out += res.results[c]["y"]
    return out
